# revision 1
# baseline (speedup 1.0000x reference)
"""Trainium2 Bass kernel for nn_CausalSelfAttention_14980845928591.

Full-input contract: kernel(**inputs) takes the unsharded numpy inputs and
returns the full [B, T, C] output. Internally shards across 8 NeuronCores:
data-parallel over B (4 batches) x tensor-parallel over heads (2 groups of 8
heads). Causal attention is independent per (batch, head); the output
projection is a partial sum over head groups, reduced on the host.

Device kernel (identical SPMD program, per-core data):
  phase 1: qT/kT (transposed layout), V (natural layout + ones column for
           softmax row-sums), gateT = sigmoid(wg^T @ xT + b) - all via
           float32r matmuls (full PE rate at N>=512, ~1e-4 rel err).
  phase 2: causal attention per head pair, S^T orientation (no transposes):
           S^T tiles = (k-block)^T x q, softmax along the free axis. Softmax without
           max-subtraction (logits are O(5) here, exp is safe in fp32);
           row-sums ride along as a 65th output row via the ones column.
  phase 3: out^T = wp^T @ (O^T * gate^T / rowsum), streamed per i-tile.
"""
import sys

sys.path.insert(0, "/opt/trn_rl_repo")

import numpy as np

import concourse.bass as bass
import concourse.mybir as mybir
import concourse.tile as tile
from concourse import bacc, bass_utils

# Problem shapes (hardcoded per contract).
B, T, C = 4, 2048, 1024
H, D = 16, 64
HL = 8            # heads per core
GC = HL * D       # 512: local q/k/v/gate/proj-row columns
P = 128
KC = C // P       # 8 contraction chunks
NTB = T // 512    # 4 T-blocks of 512
NIT = T // 512    # 4 i-tiles of 512
F32 = mybir.dt.float32
F32R = mybir.dt.float32r

_NC_CACHE = {}


def _build_nc():
    nc = bacc.Bacc("TRN2", target_bir_lowering=False, debug=False, num_devices=8)

    xT = nc.dram_tensor("xT", [C, T], F32R, kind="ExternalInput")
    wqk = nc.dram_tensor("wqk", [C, 2 * GC], F32R, kind="ExternalInput")
    wv = nc.dram_tensor("wv", [C, GC], F32R, kind="ExternalInput")
    wg = nc.dram_tensor("wg", [C, GC], F32R, kind="ExternalInput")
    bg = nc.dram_tensor("bg", [GC], F32, kind="ExternalInput")
    wp = nc.dram_tensor("wp", [GC, C], F32R, kind="ExternalInput")
    outT = nc.dram_tensor("outT", [C, T], F32, kind="ExternalOutput")
    qT_d = nc.dram_tensor("qT_scratch", [GC, T], F32R, kind="Internal")
    r_d = nc.dram_tensor("r_scratch", [NIT, 4, 1024], F32, kind="Internal")

    with tile.TileContext(nc) as tc, \
         tc.tile_pool(name="pers", bufs=1) as pers:
        # ---------- phase-1-scoped pools ----------
        with tc.tile_pool(name="psum_ph1", bufs=4, space="PSUM") as ps1, \
             tc.tile_pool(name="xs", bufs=10) as xs, \
             tc.tile_pool(name="w1", bufs=1) as w1, \
             tc.tile_pool(name="qw", bufs=3) as qwp:

            kT = [pers.tile([P, T], F32R, tag=f"kT{i}", name=f"kT{i}") for i in range(4)]
            vsb = [pers.tile([P, HL, 65], F32R, tag=f"v{j}", name=f"v{j}") for j in range(T // P)]
            gT = [pers.tile([P, T], F32, tag=f"gT{i}", name=f"gT{i}") for i in range(4)]
            wpsb = [pers.tile([P, C], F32R, tag=f"wp{k}", name=f"wp{k}") for k in range(4)]
            umask = pers.tile([P, P], F32R, tag="umask")
            zbias = pers.tile([P, 1], F32, tag="zbias")
            nc.gpsimd.memset(zbias, 0.0)
            bgsb = pers.tile([P, 4], F32, tag="bg")

            # constants
            nc.gpsimd.memset(umask.bitcast(F32), 1.0)
            # keep element where (col - row) >= 0  (upper triangular incl diag)
            nc.gpsimd.affine_select(
                out=umask.bitcast(F32), in_=umask.bitcast(F32), compare_op=mybir.AluOpType.is_ge,
                fill=0.0, base=0, pattern=[[1, P]], channel_multiplier=-1,
            )
            nc.sync.dma_start(out=bgsb, in_=bg.rearrange("(m p) -> p m", m=4))

            # weights
            wqksb = [w1.tile([P, 2 * GC], F32R, tag=f"wqk{k}", name=f"wqk{k}") for k in range(KC)]
            wvsb = [w1.tile([P, GC], F32R, tag=f"wv{k}", name=f"wv{k}") for k in range(KC)]
            wgsb = [w1.tile([P, GC], F32R, tag=f"wg{k}", name=f"wg{k}") for k in range(KC)]
            # interleave wqk with the first T-block of xT so the PE can start
            # the k-accumulation as soon as each pair of tiles lands
            xt0 = []
            for k in range(KC):
                nc.sync.dma_start(out=wqksb[k], in_=wqk[k * P:(k + 1) * P, :])
                t_ = xs.tile([P, 512], F32R, tag="xs", name="xs")
                nc.sync.dma_start(out=t_, in_=xT[k * P:(k + 1) * P, 0:512])
                xt0.append(t_)

            # ---------- phase 1: projections, per T-block ----------
            for tb in range(NTB):
                tsl = slice(tb * 512, (tb + 1) * 512)
                if tb == 0:
                    xt = xt0
                else:
                    xt = []
                    for k in range(KC):
                        t_ = xs.tile([P, 512], F32R, tag="xs", name="xs")
                        nc.sync.dma_start(out=t_, in_=xT[k * P:(k + 1) * P, tsl])
                        xt.append(t_)
                # q (m 0..3, spilled) and k (m 4..7, resident)
                for m in range(8):
                    ps = ps1.tile([P, 512], F32, tag="ps1", name="ps1")
                    for k in range(KC):
                        nc.tensor.matmul(
                            ps, wqksb[k][:, m * P:(m + 1) * P], xt[k],
                            start=(k == 0), stop=(k == KC - 1))
                    if m < 4:
                        q_ = qwp.tile([P, 512], F32R, tag="qw", name="qw")
                        nc.vector.tensor_copy(out=q_, in_=ps)
                        nc.sync.dma_start(out=qT_d[m * P:(m + 1) * P, tsl], in_=q_)
                    else:
                        nc.vector.tensor_copy(out=kT[m - 4][:, tsl], in_=ps)
                if tb == 0:
                    for k in range(KC):
                        nc.sync.dma_start(out=wvsb[k], in_=wv[k * P:(k + 1) * P, :])
                        nc.sync.dma_start(out=wgsb[k], in_=wg[k * P:(k + 1) * P, :])
                # gate (sigmoid fused on ACT)
                for m in range(4):
                    ps = ps1.tile([P, 512], F32, tag="ps1", name="ps1")
                    for k in range(KC):
                        nc.tensor.matmul(
                            ps, wgsb[k][:, m * P:(m + 1) * P], xt[k],
                            start=(k == 0), stop=(k == KC - 1))
                    nc.scalar.activation(
                        gT[m][:, tsl], ps, mybir.ActivationFunctionType.Sigmoid,
                        bias=bgsb[:, m:m + 1], scale=1.0)
                # V natural (+ones column)
                for mt in range(4):
                    j = tb * 4 + mt
                    ps = ps1.tile([P, 512], F32, tag="ps1", name="ps1")
                    for k in range(KC):
                        nc.tensor.matmul(
                            ps, xt[k][:, mt * P:(mt + 1) * P], wvsb[k],
                            start=(k == 0), stop=(k == KC - 1))
                    nc.vector.tensor_copy(
                        out=vsb[j][:, :, 0:64],
                        in_=ps.rearrange("p (h d) -> p h d", h=HL))
                    nc.vector.memset(vsb[j][:, :, 64:65].bitcast(F32), 1.0)

        for k in range(4):
            nc.sync.dma_start(out=wpsb[k], in_=wp[k * P:(k + 1) * P, :])

        # ---------- phase 2+3: attention + output projection ----------
        with tc.tile_pool(name="qs", bufs=2) as qsp, \
             tc.tile_pool(name="pT", bufs=3) as pTp, \
             tc.tile_pool(name="osb", bufs=1) as osbp, \
             tc.tile_pool(name="rr", bufs=1) as rrp, \
             tc.tile_pool(name="rbt", bufs=3) as rbtp, \
             tc.tile_pool(name="yT", bufs=2) as yTp, \
             tc.tile_pool(name="ob", bufs=4) as obp, \
             tc.tile_pool(name="ps_s", bufs=2, space="PSUM") as pss, \
             tc.tile_pool(name="ps_o", bufs=1, space="PSUM") as pso, \
             tc.tile_pool(name="ps_p", bufs=2, space="PSUM") as psp:

            for it in range(NIT):
                isl = slice(it * 512, (it + 1) * 512)
                qs = []
                for p in range(4):
                    q_ = qsp.tile([P, 512], F32R, tag=f"q{p}", name=f"q{p}")
                    nc.sync.dma_start(out=q_, in_=qT_d[p * P:(p + 1) * P, isl])
                    qs.append(q_)
                r8 = rrp.tile([P, 1024], F32, tag="r8", name="r8")
                nc.vector.memset(r8, 1.0)
                osb = []
                njb = 4 * it + 4
                for p in range(4):
                    O2 = pso.tile([65, 1024], F32, tag="O2", name="O2")
                    for jb in range(njb):
                        jsl = slice(jb * P, (jb + 1) * P)
                        s2 = pss.tile([P, 1024], F32, tag="s2", name="s2")
                        # columns below the causal diagonal block are never
                        # consumed: restrict diagonal-band matmuls/exp to the
                        # live column range [c0:512) of each head's half.
                        ko = jb - 4 * it
                        c0 = P * ko if ko > 0 else 0
                        nc.tensor.matmul(s2[:, c0:512], kT[p][0:64, jsl],
                                         qs[p][0:64, c0:512],
                                         start=True, stop=True)
                        nc.tensor.matmul(s2[:, 512 + c0:1024],
                                         kT[p][64:128, jsl],
                                         qs[p][64:128, c0:512],
                                         start=True, stop=True)
                        pT = pTp.tile([P, 1024], F32R, tag="pT", name="pT")
                        if ko > 0:
                            nc.scalar.activation(
                                pT[:, c0:512], s2[:, c0:512],
                                mybir.ActivationFunctionType.Exp,
                                bias=zbias, scale=0.125)
                            nc.scalar.activation(
                                pT[:, 512 + c0:1024], s2[:, 512 + c0:1024],
                                mybir.ActivationFunctionType.Exp,
                                bias=zbias, scale=0.125)
                        else:
                            nc.scalar.activation(
                                pT, s2, mybir.ActivationFunctionType.Exp,
                                bias=zbias, scale=0.125)
                        if ko >= 0:
                            nc.vector.tensor_mul(pT[:, c0:c0 + P],
                                                 pT[:, c0:c0 + P], umask)
                            nc.vector.tensor_mul(pT[:, 512 + c0:512 + c0 + P],
                                                 pT[:, 512 + c0:512 + c0 + P],
                                                 umask)
                        st, sp = (jb == 0), (jb == njb - 1)
                        nc.tensor.matmul(O2[:, c0:512], vsb[jb][:, 2 * p, :],
                                         pT[:, c0:512], start=st, stop=sp,
                                         skip_group_check=True)
                        nc.tensor.matmul(O2[:, 512 + c0:1024],
                                         vsb[jb][:, 2 * p + 1, :],
                                         pT[:, 512 + c0:1024],
                                         start=st, stop=sp,
                                         skip_group_check=True)
                    o_ = osbp.tile([P, 512], F32, tag=f"o{p}", name=f"o{p}")
                    nc.vector.tensor_copy(out=o_[0:64, :], in_=O2[0:64, 0:512])
                    nc.vector.tensor_copy(out=o_[64:128, :], in_=O2[0:64, 512:1024])
                    nc.vector.tensor_copy(out=r8[32 * p:32 * p + 1, :],
                                           in_=O2[64:65, :])
                    osb.append(o_)
                # reciprocal of row sums via exp(-ln(r)) on ACT; only rows
                # 0/32/64/96 hold data, the other partitions compute junk.
                r8ln = rrp.tile([P, 1024], F32, tag="r8ln", name="r8ln")
                r8rec = rrp.tile([P, 1024], F32, tag="r8rec", name="r8rec")
                nc.scalar.activation(r8ln, r8,
                                     mybir.ActivationFunctionType.Ln,
                                     bias=zbias)
                nc.scalar.activation(r8rec, r8ln,
                                     mybir.ActivationFunctionType.Exp,
                                     bias=zbias, scale=-1.0)
                for p in range(4):
                    nc.sync.dma_start(out=r_d[it, p],
                                      in_=r8rec[32 * p:32 * p + 1, :])
                # gate * recip, build yT (f32r) for the projection
                yT = [yTp.tile([P, 512], F32R, tag=f"y{p}", name=f"y{p}") for p in range(4)]
                for p in range(4):
                    rb = rbtp.tile([P, 512], F32, tag="rb", name="rb")
                    for half in range(2):
                        rsrc = r_d[it, p:p + 1, half * 512:(half + 1) * 512]
                        nc.sync.dma_start(
                            out=rb[half * 64:(half + 1) * 64, :],
                            in_=rsrc.to_broadcast((64, 512)))
                    t_ = rbtp.tile([P, 512], F32, tag="t", name="t")
                    nc.vector.tensor_mul(t_, osb[p], gT[p][:, isl])
                    nc.vector.tensor_mul(yT[p], t_, rb)
                # output projection for this i-tile
                for m in range(8):
                    ps = psp.tile([P, 512], F32, tag="po", name="po")
                    for k in range(4):
                        nc.tensor.matmul(
                            ps, wpsb[k][:, m * P:(m + 1) * P], yT[k],
                            start=(k == 0), stop=(k == 3))
                    ob = obp.tile([P, 512], F32, tag="ob", name="ob")
                    nc.vector.tensor_copy(out=ob, in_=ps)
                    nc.sync.dma_start(out=outT[m * P:(m + 1) * P, isl], in_=ob)

    nc.compile()
    return nc


def kernel(x, w_attn, w_proj, w_gate, b_gate):
    x = np.ascontiguousarray(np.asarray(x, dtype=np.float32))
    w_attn = np.asarray(w_attn, dtype=np.float32)
    w_proj = np.asarray(w_proj, dtype=np.float32)
    w_gate = np.asarray(w_gate, dtype=np.float32)
    b_gate = np.asarray(b_gate, dtype=np.float32)

    if "nc" not in _NC_CACHE:
        _NC_CACHE["nc"] = _build_nc()
    nc = _NC_CACHE["nc"]

    in_maps = []
    for c in range(8):
        b, g = c // 2, c % 2
        hsl = slice(g * GC, (g + 1) * GC)
        in_maps.append({
            "xT": np.ascontiguousarray(x[b].T),
            "wqk": np.ascontiguousarray(
                np.concatenate([w_attn[:, hsl], w_attn[:, C + g * GC:C + (g + 1) * GC]],
                               axis=1)),
            "wv": np.ascontiguousarray(w_attn[:, 2 * C + g * GC:2 * C + (g + 1) * GC]),
            "wg": np.ascontiguousarray(w_gate[:, hsl]),
            "bg": np.ascontiguousarray(b_gate[hsl]),
            "wp": np.ascontiguousarray(w_proj[hsl, :]),
        })

    res = bass_utils.run_bass_kernel_spmd(nc, in_maps, core_ids=list(range(8)))

    out = np.empty((B, T, C), dtype=np.float32)
    for b in range(B):
        acc = res.results[2 * b]["outT"].astype(np.float32)
        acc = acc + res.results[2 * b + 1]["outT"]
        out[b] = acc.T
    return out



# revision 8
# speedup vs baseline: 1.1038x; 1.1038x over previous
"""Trainium2 Bass kernel for nn_CausalSelfAttention_14980845928591.

Full-input contract: kernel(**inputs) takes the unsharded numpy inputs and
returns the full [B, T, C] output. Internally shards across 8 NeuronCores:
data-parallel over B (4 batches) x tensor-parallel over heads (2 groups of 8
heads). Causal attention is independent per (batch, head); the output
projection is a partial sum over head groups, reduced on the host.

v2 design (vs the f32r baseline):
  - all matmul operands are bf16 (host-side casts): halves the per-matmul
    stationary weight-load cost on the PE (the dominant overhead at 2.4GHz)
    and halves input DMA traffic. PSUM accumulation stays f32.
  - fully SBUF-resident: x loaded once, q kept on-chip (no DRAM spill),
    softmax row-sum reciprocals broadcast via SBUF->SBUF DMA (no DRAM
    round-trip).
  - attention inner loop software-pipelined: AV matmuls lag the S matmuls
    by two k-block iterations so the PE never stalls on the scalar-engine
    exp; masks run on gpsimd; copies split across DVE/gpsimd.
  - vector.reciprocal for row-sum reciprocals (no Ln/Exp ACT table swaps).
"""
import sys

sys.path.insert(0, "/opt/trn_rl_repo")

import numpy as np
import ml_dtypes

import concourse.bass as bass
import concourse.mybir as mybir
import concourse.tile as tile
from concourse import bacc, bass_utils

# Problem shapes (hardcoded per contract).
B, T, C = 4, 2048, 1024
H, D = 16, 64
HL = 8            # heads per core
GC = HL * D       # 512: local q/k/v/gate/proj-row columns
P = 128
KC = C // P       # 8 contraction chunks
NTB = T // 512    # 4 T-blocks of 512
NIT = T // 512    # 4 i-tiles of 512
F32 = mybir.dt.float32
BF16 = mybir.dt.bfloat16

_NC_CACHE = {}


def _build_nc():
    nc = bacc.Bacc("TRN2", target_bir_lowering=False, debug=False, num_devices=8)

    xT = nc.dram_tensor("xT", [C, T], BF16, kind="ExternalInput")
    wqk = nc.dram_tensor("wqk", [C, 2 * GC], BF16, kind="ExternalInput")
    wv = nc.dram_tensor("wv", [C, GC], BF16, kind="ExternalInput")
    wg = nc.dram_tensor("wg", [C, GC], BF16, kind="ExternalInput")
    bg = nc.dram_tensor("bg", [GC], F32, kind="ExternalInput")
    wp = nc.dram_tensor("wp", [GC, C], BF16, kind="ExternalInput")
    outT = nc.dram_tensor("outT", [C, T], F32, kind="ExternalOutput")
    r_d = nc.dram_tensor("r_scratch", [NIT, 4, 1024], F32, kind="Internal")

    with tile.TileContext(nc) as tc, \
         tc.tile_pool(name="pers", bufs=1) as pers:
        # ---------- persistent SBUF tiles ----------
        kT = [pers.tile([P, T], BF16, tag=f"kT{i}", name=f"kT{i}") for i in range(4)]
        qT = [pers.tile([P, T], BF16, tag=f"qT{i}", name=f"qT{i}") for i in range(4)]
        gT = [pers.tile([P, T], F32, tag=f"gT{i}", name=f"gT{i}") for i in range(4)]
        vsb = [pers.tile([P, HL, 65], BF16, tag=f"v{j}", name=f"v{j}")
               for j in range(T // P)]
        wpsb = [pers.tile([P, C], BF16, tag=f"wp{k}", name=f"wp{k}") for k in range(4)]
        umask = pers.tile([P, P], BF16, tag="umask")
        zbias = pers.tile([P, 1], F32, tag="zbias")
        bgsb = pers.tile([P, 4], F32, tag="bg")

        nc.gpsimd.memset(zbias, 0.0)
        nc.gpsimd.memset(umask, 1.0)
        # keep element where (col - row) >= 0  (upper triangular incl diag)
        nc.gpsimd.affine_select(
            out=umask, in_=umask, compare_op=mybir.AluOpType.is_ge,
            fill=0.0, base=0, pattern=[[1, P]], channel_multiplier=-1,
        )
        nc.sync.dma_start(out=bgsb, in_=bg.rearrange("(m p) -> p m", m=4))

        # ---------- phase 1: projections ----------
        with tc.tile_pool(name="psum_ph1", bufs=4, space="PSUM") as ps1, \
             tc.tile_pool(name="xsb", bufs=1) as xsbp, \
             tc.tile_pool(name="w1", bufs=1) as w1:

            x_sb = [xsbp.tile([P, T], BF16, tag=f"x{k}", name=f"x{k}")
                    for k in range(KC)]
            wqksb = [w1.tile([P, 2 * GC], BF16, tag=f"wqk{k}", name=f"wqk{k}")
                     for k in range(KC)]
            wvsb = [w1.tile([P, GC], BF16, tag=f"wv{k}", name=f"wv{k}")
                    for k in range(KC)]
            wgsb = [w1.tile([P, GC], BF16, tag=f"wg{k}", name=f"wg{k}")
                    for k in range(KC)]

            # prefetch order: wqk, x(tb0), wv, wg, x(tb1), wp, x(tb2), x(tb3)
            for k in range(KC):
                nc.sync.dma_start(out=wqksb[k], in_=wqk[k * P:(k + 1) * P, :])
            for k in range(KC):
                nc.sync.dma_start(out=x_sb[k][:, 0:512],
                                  in_=xT[k * P:(k + 1) * P, 0:512])
            for k in range(KC):
                nc.sync.dma_start(out=wvsb[k], in_=wv[k * P:(k + 1) * P, :])
                nc.sync.dma_start(out=wgsb[k], in_=wg[k * P:(k + 1) * P, :])
            for k in range(KC):
                nc.sync.dma_start(out=x_sb[k][:, 512:1024],
                                  in_=xT[k * P:(k + 1) * P, 512:1024])
            for k in range(4):
                nc.sync.dma_start(out=wpsb[k], in_=wp[k * P:(k + 1) * P, :])
            for tb in (2, 3):
                for k in range(KC):
                    nc.sync.dma_start(
                        out=x_sb[k][:, tb * 512:(tb + 1) * 512],
                        in_=xT[k * P:(k + 1) * P, tb * 512:(tb + 1) * 512])

            for tb in range(NTB):
                tsl = slice(tb * 512, (tb + 1) * 512)
                # q (m 0..3) and k (m 4..7), both kept transposed [gc, t]
                for m in range(8):
                    ps = ps1.tile([P, 512], F32, tag="ps1", name="ps1")
                    for k in range(KC):
                        nc.tensor.matmul(
                            ps, wqksb[k][:, m * P:(m + 1) * P], x_sb[k][:, tsl],
                            start=(k == 0), stop=(k == KC - 1))
                    if m < 4:
                        nc.vector.tensor_copy(out=qT[m][:, tsl], in_=ps)
                    else:
                        # gpsimd cannot read PSUM; ACT is idle in phase 1
                        nc.scalar.activation(
                            kT[m - 4][:, tsl], ps,
                            mybir.ActivationFunctionType.Copy)
                # gate (sigmoid fused on ACT), f32, transposed [gc, t]
                for m in range(4):
                    ps = ps1.tile([P, 512], F32, tag="ps1", name="ps1")
                    for k in range(KC):
                        nc.tensor.matmul(
                            ps, wgsb[k][:, m * P:(m + 1) * P], x_sb[k][:, tsl],
                            start=(k == 0), stop=(k == KC - 1))
                    nc.scalar.activation(
                        gT[m][:, tsl], ps, mybir.ActivationFunctionType.Sigmoid,
                        bias=bgsb[:, m:m + 1], scale=1.0)
                # V natural [t, gc] (+ones column for softmax row-sums)
                for mt in range(4):
                    j = tb * 4 + mt
                    ps = ps1.tile([P, 512], F32, tag="ps1", name="ps1")
                    for k in range(KC):
                        nc.tensor.matmul(
                            ps, x_sb[k][:, j * P:(j + 1) * P], wvsb[k],
                            start=(k == 0), stop=(k == KC - 1))
                    nc.vector.tensor_copy(
                        out=vsb[j][:, :, 0:64],
                        in_=ps.rearrange("p (h d) -> p h d", h=HL))
                    nc.gpsimd.memset(vsb[j][:, :, 64:65], 1.0)

        # ---------- phase 2+3: attention + output projection ----------
        with tc.tile_pool(name="pT", bufs=4) as pTp, \
             tc.tile_pool(name="og", bufs=2) as ogp, \
             tc.tile_pool(name="rr", bufs=2) as rrp, \
             tc.tile_pool(name="rbt", bufs=2) as rbtp, \
             tc.tile_pool(name="yT", bufs=2) as yTp, \
             tc.tile_pool(name="ob", bufs=4) as obp, \
             tc.tile_pool(name="ps_s", bufs=2, space="PSUM") as pss, \
             tc.tile_pool(name="ps_o", bufs=1, space="PSUM") as pso, \
             tc.tile_pool(name="ps_p", bufs=2, space="PSUM") as psp:

            for it in range(NIT):
                isl = slice(it * 512, (it + 1) * 512)
                njb = 4 * it + 4
                # engine partition slices must start 32-aligned, so the four
                # per-pair row-sum rows live at partitions 0/32/64/96
                r8 = rrp.tile([P, 1024], F32, tag="r8", name="r8")
                r8rec = rrp.tile([P, 1024], F32, tag="r8rec", name="r8rec")
                yts = []
                for p in range(4):
                    O2 = pso.tile([65, 1024], F32, tag="O2", name="O2")
                    # software pipeline: S(jj) runs 2 iterations ahead of
                    # AV(jj) so the PE never waits on the ACT-engine exp.
                    s2s = {}
                    pTs = {}
                    for jj in range(njb + 2):
                        if jj < njb:
                            jb = jj
                            jsl = slice(jb * P, (jb + 1) * P)
                            ko = jb - 4 * it
                            c0 = P * ko if ko > 0 else 0
                            s2 = pss.tile([P, 1024], F32, tag="s2", name="s2")
                            nc.tensor.matmul(s2[:, c0:512], kT[p][0:64, jsl],
                                             qT[p][0:64, it * 512 + c0:(it + 1) * 512],
                                             start=True, stop=True)
                            nc.tensor.matmul(s2[:, 512 + c0:1024],
                                             kT[p][64:128, jsl],
                                             qT[p][64:128, it * 512 + c0:(it + 1) * 512],
                                             start=True, stop=True)
                            pT = pTp.tile([P, 1024], BF16, tag="pT", name="pT")
                            # exp over [c0:1024): for ko>0 the dead strip
                            # [512:512+c0) holds stale-but-finite psum data;
                            # its exp lands in pT but is never consumed.
                            nc.scalar.activation(
                                pT[:, c0:1024], s2[:, c0:1024],
                                mybir.ActivationFunctionType.Exp,
                                bias=zbias, scale=0.125)
                            if ko >= 0:
                                nc.gpsimd.tensor_mul(pT[:, c0:c0 + P],
                                                     pT[:, c0:c0 + P], umask)
                                nc.gpsimd.tensor_mul(
                                    pT[:, 512 + c0:512 + c0 + P],
                                    pT[:, 512 + c0:512 + c0 + P], umask)
                            pTs[jb] = (pT, c0)
                        if jj >= 2:
                            jb = jj - 2
                            pT, c0 = pTs.pop(jb)
                            st, sp = (jb == 0), (jb == njb - 1)
                            nc.tensor.matmul(O2[:, c0:512],
                                             vsb[jb][:, 2 * p, :],
                                             pT[:, c0:512], start=st, stop=sp,
                                             skip_group_check=True)
                            nc.tensor.matmul(O2[:, 512 + c0:1024],
                                             vsb[jb][:, 2 * p + 1, :],
                                             pT[:, 512 + c0:1024],
                                             start=st, stop=sp,
                                             skip_group_check=True)
                    # O*gate, row-sum reciprocal, yT (bf16) for projection
                    og = ogp.tile([P, 512], F32, tag="og", name="og")
                    nc.vector.tensor_copy(out=og[0:64, :], in_=O2[0:64, 0:512])
                    nc.vector.tensor_copy(out=og[64:128, :], in_=O2[0:64, 512:1024])
                    nc.vector.tensor_copy(out=r8[32 * p:32 * p + 1, :],
                                          in_=O2[64:65, :])
                    nc.vector.tensor_mul(og, og, gT[p][:, isl])
                    nc.vector.reciprocal(out=r8rec[32 * p:32 * p + 1, :],
                                         in_=r8[32 * p:32 * p + 1, :])
                    # DRAM bounce: zero-step partition sources are only legal
                    # from DRAM, so round-trip the [1,1024] reciprocals.
                    nc.gpsimd.dma_start(out=r_d[it, p],
                                        in_=r8rec[32 * p:32 * p + 1, :])
                    rb = rbtp.tile([P, 512], F32, tag="rb", name="rb")
                    for half in range(2):
                        nc.gpsimd.dma_start(
                            out=rb[half * 64:(half + 1) * 64, :],
                            in_=r_d[it, p:p + 1, half * 512:(half + 1) * 512]
                            .to_broadcast((64, 512)))
                    yt = yTp.tile([P, 512], BF16, tag=f"y{p}", name=f"y{p}")
                    nc.vector.tensor_mul(yt, og, rb)
                    yts.append(yt)
                # output projection for this i-tile
                for m in range(8):
                    ps = psp.tile([P, 512], F32, tag="po", name="po")
                    for k in range(4):
                        nc.tensor.matmul(
                            ps, wpsb[k][:, m * P:(m + 1) * P], yts[k],
                            start=(k == 0), stop=(k == 3))
                    ob = obp.tile([P, 512], F32, tag="ob", name="ob")
                    nc.vector.tensor_copy(out=ob, in_=ps)
                    nc.sync.dma_start(out=outT[m * P:(m + 1) * P, isl], in_=ob)

    nc.compile()
    return nc


def make_in_maps(x, w_attn, w_proj, w_gate, b_gate):
    bf = ml_dtypes.bfloat16
    x = np.asarray(x, dtype=np.float32)
    w_attn = np.asarray(w_attn, dtype=np.float32)
    w_proj = np.asarray(w_proj, dtype=np.float32)
    w_gate = np.asarray(w_gate, dtype=np.float32)
    b_gate = np.asarray(b_gate, dtype=np.float32)
    in_maps = []
    for c in range(8):
        b, g = c // 2, c % 2
        hsl = slice(g * GC, (g + 1) * GC)
        in_maps.append({
            "xT": np.ascontiguousarray(x[b].T).astype(bf),
            "wqk": np.ascontiguousarray(
                np.concatenate([w_attn[:, hsl],
                                w_attn[:, C + g * GC:C + (g + 1) * GC]],
                               axis=1)).astype(bf),
            "wv": np.ascontiguousarray(
                w_attn[:, 2 * C + g * GC:2 * C + (g + 1) * GC]).astype(bf),
            "wg": np.ascontiguousarray(w_gate[:, hsl]).astype(bf),
            "bg": np.ascontiguousarray(b_gate[hsl]),
            "wp": np.ascontiguousarray(w_proj[hsl, :]).astype(bf),
        })
    return in_maps


def kernel(x, w_attn, w_proj, w_gate, b_gate):
    if "nc" not in _NC_CACHE:
        _NC_CACHE["nc"] = _build_nc()
    nc = _NC_CACHE["nc"]

    in_maps = make_in_maps(x, w_attn, w_proj, w_gate, b_gate)
    res = bass_utils.run_bass_kernel_spmd(nc, in_maps, core_ids=list(range(8)))

    out = np.empty((B, T, C), dtype=np.float32)
    for b in range(B):
        acc = res.results[2 * b]["outT"].astype(np.float32)
        acc = acc + res.results[2 * b + 1]["outT"]
        out[b] = acc.T
    return out


# revision 29
# speedup vs baseline: 1.1600x; 1.0510x over previous
"""Trainium2 Bass kernel for nn_CausalSelfAttention_14980845928591.

Full-input contract: kernel(**inputs) takes the unsharded numpy inputs and
returns the full [B, T, C] output. Internally shards across 8 NeuronCores:
data-parallel over B (4 batches) x tensor-parallel over heads (2 groups of 8
heads). Causal attention is independent per (batch, head); the output
projection is a partial sum over head groups, reduced on the host.

v2 design (vs the f32r baseline):
  - all matmul operands are bf16 (host-side casts): halves the per-matmul
    stationary weight-load cost on the PE (the dominant overhead at 2.4GHz)
    and halves input DMA traffic. PSUM accumulation stays f32.
  - fully SBUF-resident: x loaded once, q kept on-chip (no DRAM spill),
    softmax row-sum reciprocals broadcast via SBUF->SBUF DMA (no DRAM
    round-trip).
  - attention inner loop software-pipelined: AV matmuls lag the S matmuls
    by two k-block iterations so the PE never stalls on the scalar-engine
    exp; masks run on gpsimd; copies split across DVE/gpsimd.
  - vector.reciprocal for row-sum reciprocals (no Ln/Exp ACT table swaps).
"""
import sys

sys.path.insert(0, "/opt/trn_rl_repo")

import numpy as np
import ml_dtypes

import concourse.bass as bass
import concourse.mybir as mybir
import concourse.tile as tile
from concourse import bacc, bass_utils

# Problem shapes (hardcoded per contract).
B, T, C = 4, 2048, 1024
H, D = 16, 64
HL = 8            # heads per core
GC = HL * D       # 512: local q/k/v/gate/proj-row columns
P = 128
KC = C // P       # 8 contraction chunks
NTB = T // 512    # 4 T-blocks of 512
NIT = T // 512    # 4 i-tiles of 512
F32 = mybir.dt.float32
BF16 = mybir.dt.bfloat16

_NC_CACHE = {}


def _build_nc():
    nc = bacc.Bacc("TRN2", target_bir_lowering=False, debug=False, num_devices=8)

    xT = nc.dram_tensor("xT", [C, T], BF16, kind="ExternalInput")
    wqk = nc.dram_tensor("wqk", [C, 2 * GC], BF16, kind="ExternalInput")
    wv = nc.dram_tensor("wv", [C, GC], BF16, kind="ExternalInput")
    wg = nc.dram_tensor("wg", [C, GC], BF16, kind="ExternalInput")
    bg = nc.dram_tensor("bg", [GC], F32, kind="ExternalInput")
    wp = nc.dram_tensor("wp", [GC, C], BF16, kind="ExternalInput")
    cst = nc.dram_tensor("cst", [2, P, P], BF16, kind="ExternalInput")
    outT = nc.dram_tensor("outT", [C, T], F32, kind="ExternalOutput")
    r_d = nc.dram_tensor("r_scratch", [NIT, 4, 1024], F32, kind="Internal")
    r_d2 = nc.dram_tensor("r_scratch2", [NIT, 4, 1024], F32, kind="Internal")

    with tile.TileContext(nc) as tc, \
         tc.tile_pool(name="pers", bufs=1) as pers:
        # ---------- persistent SBUF tiles ----------
        kT = [pers.tile([P, T], BF16, tag=f"kT{i}", name=f"kT{i}") for i in range(4)]
        qT = [pers.tile([P, T], BF16, tag=f"qT{i}", name=f"qT{i}") for i in range(4)]
        gT = [pers.tile([P, T], F32, tag=f"gT{i}", name=f"gT{i}") for i in range(4)]
        vsb = [pers.tile([P, HL, 65], BF16, tag=f"v{j}", name=f"v{j}")
               for j in range(T // P)]
        wpsb = [pers.tile([P, C], BF16, tag=f"wp{k}", name=f"wp{k}") for k in range(4)]
        umask = pers.tile([P, P], BF16, tag="umask")
        zbias = pers.tile([P, 1], F32, tag="zbias")
        bgsb = pers.tile([P, 4], F32, tag="bg")

        nc.gpsimd.memset(zbias, 0.0)
        # host-built upper-triangular (incl diag) ones mask
        nc.sync.dma_start(out=umask, in_=cst[0])
        nc.sync.dma_start(out=bgsb, in_=bg.rearrange("(m p) -> p m", m=4))

        # ---------- phase 1: projections ----------
        with tc.tile_pool(name="psum_ph1", bufs=4, space="PSUM") as ps1, \
             tc.tile_pool(name="xsb", bufs=1) as xsbp, \
             tc.tile_pool(name="w1", bufs=1) as w1:

            x_sb = [xsbp.tile([P, T], BF16, tag=f"x{k}", name=f"x{k}")
                    for k in range(KC)]
            wqksb = [w1.tile([P, 2 * GC], BF16, tag=f"wqk{k}", name=f"wqk{k}")
                     for k in range(KC)]
            wvsb = [w1.tile([P, GC], BF16, tag=f"wv{k}", name=f"wv{k}")
                    for k in range(KC)]
            wgsb = [w1.tile([P, GC], BF16, tag=f"wg{k}", name=f"wg{k}")
                    for k in range(KC)]

            # prefetch order: wqk, x(tb0), wv, wg, x(tb1), wp, x(tb2), x(tb3)
            for k in range(KC):
                nc.sync.dma_start(out=wqksb[k], in_=wqk[k * P:(k + 1) * P, :])
            for k in range(KC):
                nc.sync.dma_start(out=x_sb[k][:, 0:512],
                                  in_=xT[k * P:(k + 1) * P, 0:512])
            for k in range(KC):
                nc.sync.dma_start(out=wvsb[k], in_=wv[k * P:(k + 1) * P, :])
                nc.sync.dma_start(out=wgsb[k], in_=wg[k * P:(k + 1) * P, :])
            for k in range(KC):
                nc.sync.dma_start(out=x_sb[k][:, 512:1024],
                                  in_=xT[k * P:(k + 1) * P, 512:1024])
            for k in range(4):
                nc.sync.dma_start(out=wpsb[k], in_=wp[k * P:(k + 1) * P, :])
            for tb in (2, 3):
                for k in range(KC):
                    nc.sync.dma_start(
                        out=x_sb[k][:, tb * 512:(tb + 1) * 512],
                        in_=xT[k * P:(k + 1) * P, tb * 512:(tb + 1) * 512])

            for tb in range(NTB):
                tsl = slice(tb * 512, (tb + 1) * 512)
                # q (m 0..3) and k (m 4..7), both kept transposed [gc, t]
                for m in range(8):
                    ps = ps1.tile([P, 512], F32, tag="ps1", name="ps1")
                    for k in range(KC):
                        nc.tensor.matmul(
                            ps, wqksb[k][:, m * P:(m + 1) * P], x_sb[k][:, tsl],
                            start=(k == 0), stop=(k == KC - 1))
                    if m < 4:
                        nc.vector.tensor_copy(out=qT[m][:, tsl], in_=ps)
                    else:
                        # gpsimd cannot read PSUM; ACT is idle in phase 1
                        nc.scalar.activation(
                            kT[m - 4][:, tsl], ps,
                            mybir.ActivationFunctionType.Copy)
                # gate (sigmoid fused on ACT), f32, transposed [gc, t]
                for m in range(4):
                    ps = ps1.tile([P, 512], F32, tag="ps1", name="ps1")
                    for k in range(KC):
                        nc.tensor.matmul(
                            ps, wgsb[k][:, m * P:(m + 1) * P], x_sb[k][:, tsl],
                            start=(k == 0), stop=(k == KC - 1))
                    nc.scalar.activation(
                        gT[m][:, tsl], ps, mybir.ActivationFunctionType.Sigmoid,
                        bias=bgsb[:, m:m + 1], scale=1.0)
                # V natural [t, gc] (+ones column for softmax row-sums)
                for mt in range(4):
                    j = tb * 4 + mt
                    ps = ps1.tile([P, 512], F32, tag="ps1", name="ps1")
                    for k in range(KC):
                        nc.tensor.matmul(
                            ps, x_sb[k][:, j * P:(j + 1) * P], wvsb[k],
                            start=(k == 0), stop=(k == KC - 1))
                    nc.vector.tensor_copy(
                        out=vsb[j][:, :, 0:64],
                        in_=ps.rearrange("p (h d) -> p h d", h=HL))
                    nc.gpsimd.memset(vsb[j][:, :, 64:65], 1.0)

        # ---------- phase 2+3: attention + output projection ----------
        with tc.tile_pool(name="pT", bufs=4) as pTp, \
             tc.tile_pool(name="og", bufs=2) as ogp, \
             tc.tile_pool(name="rr", bufs=2) as rrp, \
             tc.tile_pool(name="rbt", bufs=2) as rbtp, \
             tc.tile_pool(name="yT", bufs=2) as yTp, \
             tc.tile_pool(name="ob", bufs=4) as obp, \
             tc.tile_pool(name="ps_s", bufs=2, space="PSUM") as pss, \
             tc.tile_pool(name="ps_o", bufs=1, space="PSUM") as pso, \
             tc.tile_pool(name="ps_p", bufs=2, space="PSUM") as psp:

            for it in range(NIT):
                isl = slice(it * 512, (it + 1) * 512)
                njb = 4 * it + 4
                # engine partition slices must start 32-aligned, so the four
                # per-pair row-sum rows live at partitions 0/32/64/96
                r8 = rrp.tile([P, 1024], F32, tag="r8", name="r8")
                yts = []
                for p in range(4):
                    O2 = pso.tile([65, 1024], F32, tag="O2", name="O2")
                    # software pipeline: S(jj) runs 2 iterations ahead of
                    # AV(jj) so the PE never waits on the ACT-engine exp.
                    s2s = {}
                    pTs = {}
                    for jj in range(njb + 2):
                        if jj < njb:
                            jb = jj
                            jsl = slice(jb * P, (jb + 1) * P)
                            ko = jb - 4 * it
                            c0 = P * ko if ko > 0 else 0
                            s2 = pss.tile([P, 1024], F32, tag="s2", name="s2")
                            nc.tensor.matmul(s2[:, c0:512], kT[p][0:64, jsl],
                                             qT[p][0:64, it * 512 + c0:(it + 1) * 512],
                                             start=True, stop=True)
                            nc.tensor.matmul(s2[:, 512 + c0:1024],
                                             kT[p][64:128, jsl],
                                             qT[p][64:128, it * 512 + c0:(it + 1) * 512],
                                             start=True, stop=True)
                            pT = pTp.tile([P, 1024], BF16, tag="pT", name="pT")
                            # exp over [c0:1024): for ko>0 the dead strip
                            # [512:512+c0) holds stale-but-finite psum data;
                            # its exp lands in pT but is never consumed.
                            nc.scalar.activation(
                                pT[:, c0:1024], s2[:, c0:1024],
                                mybir.ActivationFunctionType.Exp,
                                bias=zbias, scale=0.125)
                            if ko >= 0:
                                # causal mask for the diagonal block
                                nc.vector.tensor_mul(pT[:, c0:c0 + P],
                                                     pT[:, c0:c0 + P], umask)
                                nc.vector.tensor_mul(
                                    pT[:, 512 + c0:512 + c0 + P],
                                    pT[:, 512 + c0:512 + c0 + P], umask)
                            pTs[jb] = (pT, c0)
                        if jj >= 2:
                            jb = jj - 2
                            pT, c0 = pTs.pop(jb)
                            st, sp = (jb == 0), (jb == njb - 1)
                            nc.tensor.matmul(O2[:, c0:512],
                                             vsb[jb][:, 2 * p, :],
                                             pT[:, c0:512], start=st, stop=sp,
                                             skip_group_check=True)
                            nc.tensor.matmul(O2[:, 512 + c0:1024],
                                             vsb[jb][:, 2 * p + 1, :],
                                             pT[:, 512 + c0:1024],
                                             start=st, stop=sp,
                                             skip_group_check=True)
                    # O*gate, row-sum reciprocal, yT (bf16) for projection
                    og = ogp.tile([P, 512], F32, tag="og", name="og")
                    nc.vector.tensor_copy(out=og[0:64, :], in_=O2[0:64, 0:512])
                    nc.vector.tensor_copy(out=og[64:128, :], in_=O2[0:64, 512:1024])
                    nc.vector.tensor_copy(out=r8[32 * p:32 * p + 1, :],
                                          in_=O2[64:65, :])
                    nc.vector.tensor_mul(og, og, gT[p][:, isl])
                    # Row-sum reciprocals: the [1,1024] row would cost ~6.5us
                    # on the DVE (cost is free-size bound), so bounce through
                    # DRAM and reshape to [32,32] to make the reciprocal
                    # ~free, then bounce again for the partition-broadcast.
                    nc.sync.dma_start(out=r_d[it, p],
                                      in_=r8[32 * p:32 * p + 1, :])
                    rq = rrp.tile([32, 32], F32, tag="rq", name="rq")
                    rqr = rrp.tile([32, 32], F32, tag="rqr", name="rqr")
                    nc.sync.dma_start(
                        out=rq, in_=r_d[it, p].rearrange("(a b) -> a b", a=32))
                    nc.vector.reciprocal(out=rqr, in_=rq)
                    nc.sync.dma_start(
                        out=r_d2[it, p].rearrange("(a b) -> a b", a=32),
                        in_=rqr)
                    rb = rbtp.tile([P, 512], F32, tag="rb", name="rb")
                    for half in range(2):
                        nc.sync.dma_start(
                            out=rb[half * 64:(half + 1) * 64, :],
                            in_=r_d2[it, p:p + 1, half * 512:(half + 1) * 512]
                            .to_broadcast((64, 512)))
                    yt = yTp.tile([P, 512], BF16, tag=f"y{p}", name=f"y{p}")
                    nc.vector.tensor_mul(yt, og, rb)
                    yts.append(yt)
                # output projection for this i-tile
                for m in range(8):
                    ps = psp.tile([P, 512], F32, tag="po", name="po")
                    for k in range(4):
                        nc.tensor.matmul(
                            ps, wpsb[k][:, m * P:(m + 1) * P], yts[k],
                            start=(k == 0), stop=(k == 3))
                    ob = obp.tile([P, 512], F32, tag="ob", name="ob")
                    nc.vector.tensor_copy(out=ob, in_=ps)
                    nc.sync.dma_start(out=outT[m * P:(m + 1) * P, isl], in_=ob)

    nc.compile()
    return nc


def make_in_maps(x, w_attn, w_proj, w_gate, b_gate):
    bf = ml_dtypes.bfloat16
    umask_np = np.triu(np.ones((P, P), dtype=np.float32))
    cst = np.ascontiguousarray(
        np.stack([umask_np, np.zeros((P, P), np.float32)])).astype(bf)
    x = np.asarray(x, dtype=np.float32)
    w_attn = np.asarray(w_attn, dtype=np.float32)
    w_proj = np.asarray(w_proj, dtype=np.float32)
    w_gate = np.asarray(w_gate, dtype=np.float32)
    b_gate = np.asarray(b_gate, dtype=np.float32)
    in_maps = []
    for c in range(8):
        b, g = c // 2, c % 2
        hsl = slice(g * GC, (g + 1) * GC)
        in_maps.append({
            "xT": np.ascontiguousarray(x[b].T).astype(bf),
            "wqk": np.ascontiguousarray(
                np.concatenate([w_attn[:, hsl],
                                w_attn[:, C + g * GC:C + (g + 1) * GC]],
                               axis=1)).astype(bf),
            "wv": np.ascontiguousarray(
                w_attn[:, 2 * C + g * GC:2 * C + (g + 1) * GC]).astype(bf),
            "wg": np.ascontiguousarray(w_gate[:, hsl]).astype(bf),
            "bg": np.ascontiguousarray(b_gate[hsl]),
            "wp": np.ascontiguousarray(w_proj[hsl, :]).astype(bf),
            "cst": cst,
        })
    return in_maps


def kernel(x, w_attn, w_proj, w_gate, b_gate):
    if "nc" not in _NC_CACHE:
        _NC_CACHE["nc"] = _build_nc()
    nc = _NC_CACHE["nc"]

    in_maps = make_in_maps(x, w_attn, w_proj, w_gate, b_gate)
    res = bass_utils.run_bass_kernel_spmd(nc, in_maps, core_ids=list(range(8)))

    out = np.empty((B, T, C), dtype=np.float32)
    for b in range(B):
        acc = res.results[2 * b]["outT"].astype(np.float32)
        acc = acc + res.results[2 * b + 1]["outT"]
        out[b] = acc.T
    return out


# revision 30
# speedup vs baseline: 1.4909x; 1.2852x over previous
"""Trainium2 Bass kernel for nn_CausalSelfAttention_14980845928591.

Full-input contract: kernel(**inputs) takes the unsharded numpy inputs and
returns the full [B, T, C] output. Internally shards across 8 NeuronCores:
data-parallel over B (4 batches) x tensor-parallel over heads (2 groups of 8
heads). Causal attention is independent per (batch, head); the output
projection is a partial sum over head groups, reduced on the host.

v4 design:
  - all matmul operands bf16 (host-side casts): the PE hides bf16 weight
    loads behind streaming (fp32r self-loading matmuls cannot); PSUM
    accumulation stays f32.
  - fully SBUF-resident: x loaded once, q kept on-chip.
  - projection rounds interleaved with attention i-tiles: attention's exp
    stream (the scalar-engine bottleneck) overlaps the ACT-free QKV/V
    matmul blocks of the next t-block round.
  - attention inner loop software-pipelined: AV matmuls lag the S matmuls
    by two k-block iterations so the PE rarely waits on exp.
  - softmax row-sum reciprocals via a DRAM reshape bounce ([1,1024] ->
    [32,32]) making the DVE reciprocal ~free; partition-broadcast of the
    reciprocals also via DRAM (zero-step partition DMA needs a DRAM src).
"""
import sys

sys.path.insert(0, "/opt/trn_rl_repo")

import numpy as np
import ml_dtypes

import concourse.bass as bass
import concourse.mybir as mybir
import concourse.tile as tile
from concourse import bacc, bass_utils

# Problem shapes (hardcoded per contract).
B, T, C = 4, 2048, 1024
H, D = 16, 64
HL = 8            # heads per core
GC = HL * D       # 512: local q/k/v/gate/proj-row columns
P = 128
KC = C // P       # 8 contraction chunks
NTB = T // 512    # 4 T-blocks of 512
NIT = T // 512    # 4 i-tiles of 512
F32 = mybir.dt.float32
BF16 = mybir.dt.bfloat16

_NC_CACHE = {}


def _build_nc():
    nc = bacc.Bacc("TRN2", target_bir_lowering=False, debug=False, num_devices=8)

    xT = nc.dram_tensor("xT", [C, T], BF16, kind="ExternalInput")
    wqk = nc.dram_tensor("wqk", [C, 2 * GC], BF16, kind="ExternalInput")
    wv = nc.dram_tensor("wv", [C, GC], BF16, kind="ExternalInput")
    wg = nc.dram_tensor("wg", [C, GC], BF16, kind="ExternalInput")
    bg = nc.dram_tensor("bg", [GC], F32, kind="ExternalInput")
    wp = nc.dram_tensor("wp", [GC, C], BF16, kind="ExternalInput")
    cst = nc.dram_tensor("cst", [2, P, P], BF16, kind="ExternalInput")
    outT = nc.dram_tensor("outT", [C, T], F32, kind="ExternalOutput")
    r_d = nc.dram_tensor("r_scratch", [NIT, 4, 1024], F32, kind="Internal")
    r_d2 = nc.dram_tensor("r_scratch2", [NIT, 4, 1024], F32, kind="Internal")

    with tile.TileContext(nc) as tc, \
         tc.tile_pool(name="pers", bufs=1) as pers, \
         tc.tile_pool(name="xsb", bufs=1) as xsbp, \
         tc.tile_pool(name="w1", bufs=1) as w1, \
         tc.tile_pool(name="pT", bufs=3) as pTp, \
         tc.tile_pool(name="og", bufs=2) as ogp, \
         tc.tile_pool(name="rr", bufs=2) as rrp, \
         tc.tile_pool(name="rbt", bufs=2) as rbtp, \
         tc.tile_pool(name="yT", bufs=2) as yTp, \
         tc.tile_pool(name="ob", bufs=2) as obp, \
         tc.tile_pool(name="ps1", bufs=2, space="PSUM") as ps1, \
         tc.tile_pool(name="ps_s", bufs=2, space="PSUM") as pss, \
         tc.tile_pool(name="ps_o", bufs=1, space="PSUM") as pso:

        # ---------- persistent SBUF tiles ----------
        kT = [pers.tile([P, T], BF16, tag=f"kT{i}", name=f"kT{i}") for i in range(4)]
        qT = [pers.tile([P, T], BF16, tag=f"qT{i}", name=f"qT{i}") for i in range(4)]
        gT = [pers.tile([P, T], F32, tag=f"gT{i}", name=f"gT{i}") for i in range(4)]
        vsb = [pers.tile([P, HL, 65], BF16, tag=f"v{j}", name=f"v{j}")
               for j in range(T // P)]
        wpsb = [pers.tile([P, C], BF16, tag=f"wp{k}", name=f"wp{k}") for k in range(4)]
        umask = pers.tile([P, P], BF16, tag="umask")
        zbias = pers.tile([P, 1], F32, tag="zbias")
        bgsb = pers.tile([P, 4], F32, tag="bg")
        x_sb = [xsbp.tile([P, T], BF16, tag=f"x{k}", name=f"x{k}")
                for k in range(KC)]
        wqksb = [w1.tile([P, 2 * GC], BF16, tag=f"wqk{k}", name=f"wqk{k}")
                 for k in range(KC)]
        wvsb = [w1.tile([P, GC], BF16, tag=f"wv{k}", name=f"wv{k}")
                for k in range(KC)]
        wgsb = [w1.tile([P, GC], BF16, tag=f"wg{k}", name=f"wg{k}")
                for k in range(KC)]

        nc.gpsimd.memset(zbias, 0.0)
        nc.sync.dma_start(out=umask, in_=cst[0])
        nc.sync.dma_start(out=bgsb, in_=bg.rearrange("(m p) -> p m", m=4))

        # prefetch order: wqk, x(tb0), wv, wg, x(tb1), wp, x(tb2), x(tb3)
        for k in range(KC):
            nc.sync.dma_start(out=wqksb[k], in_=wqk[k * P:(k + 1) * P, :])
        for k in range(KC):
            nc.sync.dma_start(out=x_sb[k][:, 0:512],
                              in_=xT[k * P:(k + 1) * P, 0:512])
        for k in range(KC):
            nc.sync.dma_start(out=wvsb[k], in_=wv[k * P:(k + 1) * P, :])
            nc.sync.dma_start(out=wgsb[k], in_=wg[k * P:(k + 1) * P, :])
        for k in range(KC):
            nc.sync.dma_start(out=x_sb[k][:, 512:1024],
                              in_=xT[k * P:(k + 1) * P, 512:1024])
        for k in range(4):
            nc.sync.dma_start(out=wpsb[k], in_=wp[k * P:(k + 1) * P, :])
        for tb in (2, 3):
            for k in range(KC):
                nc.sync.dma_start(
                    out=x_sb[k][:, tb * 512:(tb + 1) * 512],
                    in_=xT[k * P:(k + 1) * P, tb * 512:(tb + 1) * 512])

        # ---------- phase-1 building blocks ----------
        def qk_block(tb, m):
            tsl = slice(tb * 512, (tb + 1) * 512)
            ps = ps1.tile([P, 512], F32, tag="ps1", name="ps1")
            for k in range(KC):
                nc.tensor.matmul(
                    ps, wqksb[k][:, m * P:(m + 1) * P], x_sb[k][:, tsl],
                    start=(k == 0), stop=(k == KC - 1))
            if m < 4:
                nc.vector.tensor_copy(out=qT[m][:, tsl], in_=ps)
            else:
                nc.vector.tensor_copy(out=kT[m - 4][:, tsl], in_=ps)

        def gate_block(tb, m):
            tsl = slice(tb * 512, (tb + 1) * 512)
            ps = ps1.tile([P, 512], F32, tag="ps1", name="ps1")
            for k in range(KC):
                nc.tensor.matmul(
                    ps, wgsb[k][:, m * P:(m + 1) * P], x_sb[k][:, tsl],
                    start=(k == 0), stop=(k == KC - 1))
            nc.scalar.activation(
                gT[m][:, tsl], ps, mybir.ActivationFunctionType.Sigmoid,
                bias=bgsb[:, m:m + 1], scale=1.0)

        def v_block(tb, mt):
            j = tb * 4 + mt
            ps = ps1.tile([P, 512], F32, tag="ps1", name="ps1")
            for k in range(KC):
                nc.tensor.matmul(
                    ps, x_sb[k][:, j * P:(j + 1) * P], wvsb[k],
                    start=(k == 0), stop=(k == KC - 1))
            nc.vector.tensor_copy(
                out=vsb[j][:, :, 0:64],
                in_=ps.rearrange("p (h d) -> p h d", h=HL))
            nc.gpsimd.memset(vsb[j][:, :, 64:65], 1.0)

        # ---------- attention building blocks ----------
        def attn_p(it, p, r8):
            isl = slice(it * 512, (it + 1) * 512)
            njb = 4 * it + 4
            O2 = pso.tile([65, 1024], F32, tag="O2", name="O2")
            # software pipeline: S(jj) runs 2 iterations ahead of AV(jj)
            # so the PE rarely waits on the ACT-engine exp.
            pTs = {}
            for jj in range(njb + 2):
                if jj < njb:
                    jb = jj
                    jsl = slice(jb * P, (jb + 1) * P)
                    ko = jb - 4 * it
                    c0 = P * ko if ko > 0 else 0
                    s2 = pss.tile([P, 1024], F32, tag="s2", name="s2")
                    nc.tensor.matmul(s2[:, c0:512], kT[p][0:64, jsl],
                                     qT[p][0:64, it * 512 + c0:(it + 1) * 512],
                                     start=True, stop=True)
                    nc.tensor.matmul(s2[:, 512 + c0:1024],
                                     kT[p][64:128, jsl],
                                     qT[p][64:128, it * 512 + c0:(it + 1) * 512],
                                     start=True, stop=True)
                    pT = pTp.tile([P, 1024], BF16, tag="pT", name="pT")
                    # exp over [c0:1024): for ko>0 the dead strip
                    # [512:512+c0) holds stale-but-finite psum data; its
                    # exp lands in pT but is never consumed.
                    nc.scalar.activation(
                        pT[:, c0:1024], s2[:, c0:1024],
                        mybir.ActivationFunctionType.Exp,
                        bias=zbias, scale=0.125)
                    if ko >= 0:
                        # causal mask for the diagonal block
                        nc.vector.tensor_mul(pT[:, c0:c0 + P],
                                             pT[:, c0:c0 + P], umask)
                        nc.vector.tensor_mul(pT[:, 512 + c0:512 + c0 + P],
                                             pT[:, 512 + c0:512 + c0 + P],
                                             umask)
                    pTs[jb] = (pT, c0)
                if jj >= 2:
                    jb = jj - 2
                    pT, c0 = pTs.pop(jb)
                    st, sp = (jb == 0), (jb == njb - 1)
                    nc.tensor.matmul(O2[:, c0:512], vsb[jb][:, 2 * p, :],
                                     pT[:, c0:512], start=st, stop=sp,
                                     skip_group_check=True)
                    nc.tensor.matmul(O2[:, 512 + c0:1024],
                                     vsb[jb][:, 2 * p + 1, :],
                                     pT[:, 512 + c0:1024],
                                     start=st, stop=sp,
                                     skip_group_check=True)
            # O*gate (folded into the PSUM read), row-sum reciprocal, yT
            og = ogp.tile([P, 512], F32, tag="og", name="og")
            nc.vector.tensor_mul(og[0:64, :], O2[0:64, 0:512],
                                 gT[p][0:64, isl])
            nc.vector.tensor_mul(og[64:128, :], O2[0:64, 512:1024],
                                 gT[p][64:128, isl])
            nc.vector.tensor_copy(out=r8[32 * p:32 * p + 1, :],
                                  in_=O2[64:65, :])
            # Row-sum reciprocals: a [1,1024] DVE reciprocal costs ~6.5us
            # (cost is free-size bound), so bounce through DRAM reshaped to
            # [32,32], then bounce again for the partition-broadcast.
            nc.gpsimd.dma_start(out=r_d[it, p],
                                in_=r8[32 * p:32 * p + 1, :])
            rq = rrp.tile([32, 32], F32, tag="rq", name="rq")
            rqr = rrp.tile([32, 32], F32, tag="rqr", name="rqr")
            nc.gpsimd.dma_start(
                out=rq, in_=r_d[it, p].rearrange("(a b) -> a b", a=32))
            nc.vector.reciprocal(out=rqr, in_=rq)
            nc.gpsimd.dma_start(
                out=r_d2[it, p].rearrange("(a b) -> a b", a=32), in_=rqr)
            rb = rbtp.tile([P, 512], F32, tag="rb", name="rb")
            for half in range(2):
                nc.gpsimd.dma_start(
                    out=rb[half * 64:(half + 1) * 64, :],
                    in_=r_d2[it, p:p + 1, half * 512:(half + 1) * 512]
                    .to_broadcast((64, 512)))
            yt = yTp.tile([P, 512], BF16, tag=f"y{p}", name=f"y{p}")
            nc.vector.tensor_mul(yt, og, rb)
            return yt

        def proj_it(it, yts):
            isl = slice(it * 512, (it + 1) * 512)
            for m in range(8):
                ps = ps1.tile([P, 512], F32, tag="ps1", name="ps1")
                for k in range(4):
                    nc.tensor.matmul(
                        ps, wpsb[k][:, m * P:(m + 1) * P], yts[k],
                        start=(k == 0), stop=(k == 3))
                ob = obp.tile([P, 512], F32, tag="ob", name="ob")
                nc.vector.tensor_copy(out=ob, in_=ps)
                nc.sync.dma_start(out=outT[m * P:(m + 1) * P, isl], in_=ob)

        # ---------- schedule ----------
        # round 0: QK/V of tb0 + ALL gate blocks (keeps the ACT sigmoid
        # burst before the exp stream starts: one table swap total)
        for m in range(8):
            qk_block(0, m)
        for mt in range(4):
            v_block(0, mt)
        for tb in range(NTB):
            for m in range(4):
                gate_block(tb, m)
        # rounds 1..3: tb r projections interleaved with attention it r-1;
        # round 4: attention it3
        for r in range(1, 5):
            it = r - 1
            r8 = rrp.tile([P, 1024], F32, tag="r8", name="r8")
            yts = []
            for p in range(4):
                if r <= 3:
                    qk_block(r, 2 * p)
                    qk_block(r, 2 * p + 1)
                yts.append(attn_p(it, p, r8))
            if r <= 3:
                for mt in range(4):
                    v_block(r, mt)
            proj_it(it, yts)

    nc.compile()
    return nc


def make_in_maps(x, w_attn, w_proj, w_gate, b_gate):
    bf = ml_dtypes.bfloat16
    umask_np = np.triu(np.ones((P, P), dtype=np.float32))
    cst = np.ascontiguousarray(
        np.stack([umask_np, np.zeros((P, P), np.float32)])).astype(bf)
    x = np.asarray(x, dtype=np.float32)
    w_attn = np.asarray(w_attn, dtype=np.float32)
    w_proj = np.asarray(w_proj, dtype=np.float32)
    w_gate = np.asarray(w_gate, dtype=np.float32)
    b_gate = np.asarray(b_gate, dtype=np.float32)
    in_maps = []
    for c in range(8):
        b, g = c // 2, c % 2
        hsl = slice(g * GC, (g + 1) * GC)
        in_maps.append({
            "xT": np.ascontiguousarray(x[b].T).astype(bf),
            "wqk": np.ascontiguousarray(
                np.concatenate([w_attn[:, hsl],
                                w_attn[:, C + g * GC:C + (g + 1) * GC]],
                               axis=1)).astype(bf),
            "wv": np.ascontiguousarray(
                w_attn[:, 2 * C + g * GC:2 * C + (g + 1) * GC]).astype(bf),
            "wg": np.ascontiguousarray(w_gate[:, hsl]).astype(bf),
            "bg": np.ascontiguousarray(b_gate[hsl]),
            "wp": np.ascontiguousarray(w_proj[hsl, :]).astype(bf),
            "cst": cst,
        })
    return in_maps


def kernel(x, w_attn, w_proj, w_gate, b_gate):
    if "nc" not in _NC_CACHE:
        _NC_CACHE["nc"] = _build_nc()
    nc = _NC_CACHE["nc"]

    in_maps = make_in_maps(x, w_attn, w_proj, w_gate, b_gate)
    res = bass_utils.run_bass_kernel_spmd(nc, in_maps, core_ids=list(range(8)))

    out = np.empty((B, T, C), dtype=np.float32)
    for b in range(B):
        acc = res.results[2 * b]["outT"].astype(np.float32)
        acc = acc + res.results[2 * b + 1]["outT"]
        out[b] = acc.T
    return out


# revision 32
# speedup vs baseline: 1.5327x; 1.0281x over previous
"""Trainium2 Bass kernel for nn_CausalSelfAttention_14980845928591.

Full-input contract: kernel(**inputs) takes the unsharded numpy inputs and
returns the full [B, T, C] output. Internally shards across 8 NeuronCores:
data-parallel over B (4 batches) x tensor-parallel over heads (2 groups of 8
heads). Causal attention is independent per (batch, head); the output
projection is a partial sum over head groups, reduced on the host.

v4 design:
  - all matmul operands bf16 (host-side casts): the PE hides bf16 weight
    loads behind streaming (fp32r self-loading matmuls cannot); PSUM
    accumulation stays f32.
  - fully SBUF-resident: x loaded once, q kept on-chip.
  - projection rounds interleaved with attention i-tiles: attention's exp
    stream (the scalar-engine bottleneck) overlaps the ACT-free QKV/V
    matmul blocks of the next t-block round.
  - attention inner loop software-pipelined: AV matmuls lag the S matmuls
    by two k-block iterations so the PE rarely waits on exp.
  - softmax row-sum reciprocals via a DRAM reshape bounce ([1,1024] ->
    [32,32]) making the DVE reciprocal ~free; partition-broadcast of the
    reciprocals also via DRAM (zero-step partition DMA needs a DRAM src).
"""
import sys

sys.path.insert(0, "/opt/trn_rl_repo")

import numpy as np
import ml_dtypes

import concourse.bass as bass
import concourse.mybir as mybir
import concourse.tile as tile
from concourse import bacc, bass_utils

# Problem shapes (hardcoded per contract).
B, T, C = 4, 2048, 1024
H, D = 16, 64
HL = 8            # heads per core
GC = HL * D       # 512: local q/k/v/gate/proj-row columns
P = 128
KC = C // P       # 8 contraction chunks
NTB = T // 512    # 4 T-blocks of 512
NIT = T // 512    # 4 i-tiles of 512
F32 = mybir.dt.float32
BF16 = mybir.dt.bfloat16

_NC_CACHE = {}


def _build_nc():
    nc = bacc.Bacc("TRN2", target_bir_lowering=False, debug=False, num_devices=8)

    xT = nc.dram_tensor("xT", [C, T], BF16, kind="ExternalInput")
    wqk = nc.dram_tensor("wqk", [C, 2 * GC], BF16, kind="ExternalInput")
    wv = nc.dram_tensor("wv", [C, GC], BF16, kind="ExternalInput")
    wg = nc.dram_tensor("wg", [C, GC], BF16, kind="ExternalInput")
    bg = nc.dram_tensor("bg", [GC], F32, kind="ExternalInput")
    wp = nc.dram_tensor("wp", [GC, C], BF16, kind="ExternalInput")
    cst = nc.dram_tensor("cst", [2, P, P], BF16, kind="ExternalInput")
    outT = nc.dram_tensor("outT", [C, T], F32, kind="ExternalOutput")
    r_d = nc.dram_tensor("r_scratch", [NIT, 4, 1024], F32, kind="Internal")
    r_d2 = nc.dram_tensor("r_scratch2", [NIT, 4, 1024], F32, kind="Internal")

    with tile.TileContext(nc) as tc, \
         tc.tile_pool(name="pers", bufs=1) as pers, \
         tc.tile_pool(name="xsb", bufs=1) as xsbp, \
         tc.tile_pool(name="w1", bufs=1) as w1, \
         tc.tile_pool(name="pT", bufs=3) as pTp, \
         tc.tile_pool(name="og", bufs=2) as ogp, \
         tc.tile_pool(name="rr", bufs=2) as rrp, \
         tc.tile_pool(name="rbt", bufs=2) as rbtp, \
         tc.tile_pool(name="yT", bufs=2) as yTp, \
         tc.tile_pool(name="ob", bufs=2) as obp, \
         tc.tile_pool(name="ps1", bufs=2, space="PSUM") as ps1, \
         tc.tile_pool(name="ps_s", bufs=2, space="PSUM") as pss, \
         tc.tile_pool(name="ps_o", bufs=1, space="PSUM") as pso:

        # ---------- persistent SBUF tiles ----------
        kT = [pers.tile([P, T], BF16, tag=f"kT{i}", name=f"kT{i}") for i in range(4)]
        qT = [pers.tile([P, T], BF16, tag=f"qT{i}", name=f"qT{i}") for i in range(4)]
        gT = [pers.tile([P, T], F32, tag=f"gT{i}", name=f"gT{i}") for i in range(4)]
        vsb = [pers.tile([P, HL, 65], BF16, tag=f"v{j}", name=f"v{j}")
               for j in range(T // P)]
        wpsb = [pers.tile([P, C], BF16, tag=f"wp{k}", name=f"wp{k}") for k in range(4)]
        umask = pers.tile([P, P], BF16, tag="umask")
        zbias = pers.tile([P, 1], F32, tag="zbias")
        bgsb = pers.tile([P, 4], F32, tag="bg")
        x_sb = [xsbp.tile([P, T], BF16, tag=f"x{k}", name=f"x{k}")
                for k in range(KC)]
        wqksb = [w1.tile([P, 2 * GC], BF16, tag=f"wqk{k}", name=f"wqk{k}")
                 for k in range(KC)]
        wvsb = [w1.tile([P, GC], BF16, tag=f"wv{k}", name=f"wv{k}")
                for k in range(KC)]
        wgsb = [w1.tile([P, GC], BF16, tag=f"wg{k}", name=f"wg{k}")
                for k in range(KC)]

        nc.gpsimd.memset(zbias, 0.0)
        nc.sync.dma_start(out=umask, in_=cst[0])
        nc.sync.dma_start(out=bgsb, in_=bg.rearrange("(m p) -> p m", m=4))

        # prefetch order: wqk, x(tb0), wv, wg, x(tb1), wp, x(tb2), x(tb3)
        for k in range(KC):
            nc.sync.dma_start(out=wqksb[k], in_=wqk[k * P:(k + 1) * P, :])
        for k in range(KC):
            nc.sync.dma_start(out=x_sb[k][:, 0:512],
                              in_=xT[k * P:(k + 1) * P, 0:512])
        for k in range(KC):
            nc.sync.dma_start(out=wvsb[k], in_=wv[k * P:(k + 1) * P, :])
            nc.sync.dma_start(out=wgsb[k], in_=wg[k * P:(k + 1) * P, :])
        for k in range(KC):
            nc.sync.dma_start(out=x_sb[k][:, 512:1024],
                              in_=xT[k * P:(k + 1) * P, 512:1024])
        for k in range(4):
            nc.sync.dma_start(out=wpsb[k], in_=wp[k * P:(k + 1) * P, :])
        for tb in (2, 3):
            for k in range(KC):
                nc.sync.dma_start(
                    out=x_sb[k][:, tb * 512:(tb + 1) * 512],
                    in_=xT[k * P:(k + 1) * P, tb * 512:(tb + 1) * 512])

        # ---------- phase-1 building blocks ----------
        def qk_block(tb, m):
            tsl = slice(tb * 512, (tb + 1) * 512)
            ps = ps1.tile([P, 512], F32, tag="ps1", name="ps1")
            for k in range(KC):
                nc.tensor.matmul(
                    ps, wqksb[k][:, m * P:(m + 1) * P], x_sb[k][:, tsl],
                    start=(k == 0), stop=(k == KC - 1))
            if m < 4:
                nc.vector.tensor_copy(out=qT[m][:, tsl], in_=ps)
            else:
                nc.vector.tensor_copy(out=kT[m - 4][:, tsl], in_=ps)

        def gate_block(tb, m):
            tsl = slice(tb * 512, (tb + 1) * 512)
            ps = ps1.tile([P, 512], F32, tag="ps1", name="ps1")
            for k in range(KC):
                nc.tensor.matmul(
                    ps, wgsb[k][:, m * P:(m + 1) * P], x_sb[k][:, tsl],
                    start=(k == 0), stop=(k == KC - 1))
            nc.scalar.activation(
                gT[m][:, tsl], ps, mybir.ActivationFunctionType.Sigmoid,
                bias=bgsb[:, m:m + 1], scale=1.0)

        def v_block(tb, mt):
            j = tb * 4 + mt
            ps = ps1.tile([P, 512], F32, tag="ps1", name="ps1")
            for k in range(KC):
                nc.tensor.matmul(
                    ps, x_sb[k][:, j * P:(j + 1) * P], wvsb[k],
                    start=(k == 0), stop=(k == KC - 1))
            nc.vector.tensor_copy(
                out=vsb[j][:, :, 0:64],
                in_=ps.rearrange("p (h d) -> p h d", h=HL))
            nc.gpsimd.memset(vsb[j][:, :, 64:65], 1.0)

        # ---------- attention building blocks ----------
        def attn_p(it, p, r8):
            isl = slice(it * 512, (it + 1) * 512)
            njb = 4 * it + 4
            O2 = pso.tile([65, 1024], F32, tag="O2", name="O2")
            # software pipeline: S(jj) runs 2 iterations ahead of AV(jj)
            # so the PE rarely waits on the ACT-engine exp.
            pTs = {}
            for jj in range(njb + 2):
                if jj < njb:
                    jb = jj
                    jsl = slice(jb * P, (jb + 1) * P)
                    ko = jb - 4 * it
                    c0 = P * ko if ko > 0 else 0
                    s2 = pss.tile([P, 1024], F32, tag="s2", name="s2")
                    nc.tensor.matmul(s2[:, c0:512], kT[p][0:64, jsl],
                                     qT[p][0:64, it * 512 + c0:(it + 1) * 512],
                                     start=True, stop=True)
                    nc.tensor.matmul(s2[:, 512 + c0:1024],
                                     kT[p][64:128, jsl],
                                     qT[p][64:128, it * 512 + c0:(it + 1) * 512],
                                     start=True, stop=True)
                    pT = pTp.tile([P, 1024], BF16, tag="pT", name="pT")
                    if ko > 0:
                        # one ACT call over both heads' live column blocks
                        # via a strided view; skips the dead strip between
                        s2v = s2.rearrange("p (h t) -> p h t", h=2)
                        pTv = pT.rearrange("p (h t) -> p h t", h=2)
                        nc.scalar.activation(
                            pTv[:, :, c0:512], s2v[:, :, c0:512],
                            mybir.ActivationFunctionType.Exp,
                            bias=zbias, scale=0.125)
                    else:
                        nc.scalar.activation(
                            pT, s2, mybir.ActivationFunctionType.Exp,
                            bias=zbias, scale=0.125)
                    if ko >= 0:
                        # causal mask for the diagonal block
                        nc.vector.tensor_mul(pT[:, c0:c0 + P],
                                             pT[:, c0:c0 + P], umask)
                        nc.vector.tensor_mul(pT[:, 512 + c0:512 + c0 + P],
                                             pT[:, 512 + c0:512 + c0 + P],
                                             umask)
                    pTs[jb] = (pT, c0)
                if jj >= 2:
                    jb = jj - 2
                    pT, c0 = pTs.pop(jb)
                    st, sp = (jb == 0), (jb == njb - 1)
                    nc.tensor.matmul(O2[:, c0:512], vsb[jb][:, 2 * p, :],
                                     pT[:, c0:512], start=st, stop=sp,
                                     skip_group_check=True)
                    nc.tensor.matmul(O2[:, 512 + c0:1024],
                                     vsb[jb][:, 2 * p + 1, :],
                                     pT[:, 512 + c0:1024],
                                     start=st, stop=sp,
                                     skip_group_check=True)
            # O*gate (folded into the PSUM read), row-sum reciprocal, yT
            og = ogp.tile([P, 512], F32, tag="og", name="og")
            nc.vector.tensor_mul(og[0:64, :], O2[0:64, 0:512],
                                 gT[p][0:64, isl])
            nc.vector.tensor_mul(og[64:128, :], O2[0:64, 512:1024],
                                 gT[p][64:128, isl])
            nc.vector.tensor_copy(out=r8[32 * p:32 * p + 1, :],
                                  in_=O2[64:65, :])
            # Row-sum reciprocals: a [1,1024] DVE reciprocal costs ~6.5us
            # (cost is free-size bound), so bounce through DRAM reshaped to
            # [32,32], then bounce again for the partition-broadcast.
            nc.gpsimd.dma_start(out=r_d[it, p],
                                in_=r8[32 * p:32 * p + 1, :])
            rq = rrp.tile([32, 32], F32, tag="rq", name="rq")
            rqr = rrp.tile([32, 32], F32, tag="rqr", name="rqr")
            nc.gpsimd.dma_start(
                out=rq, in_=r_d[it, p].rearrange("(a b) -> a b", a=32))
            nc.vector.reciprocal(out=rqr, in_=rq)
            nc.gpsimd.dma_start(
                out=r_d2[it, p].rearrange("(a b) -> a b", a=32), in_=rqr)
            rb = rbtp.tile([P, 512], F32, tag="rb", name="rb")
            for half in range(2):
                nc.gpsimd.dma_start(
                    out=rb[half * 64:(half + 1) * 64, :],
                    in_=r_d2[it, p:p + 1, half * 512:(half + 1) * 512]
                    .to_broadcast((64, 512)))
            yt = yTp.tile([P, 512], BF16, tag=f"y{p}", name=f"y{p}")
            nc.vector.tensor_mul(yt, og, rb)
            return yt

        def proj_m(it, yts, m):
            isl = slice(it * 512, (it + 1) * 512)
            ps = ps1.tile([P, 512], F32, tag="ps1", name="ps1")
            for k in range(4):
                nc.tensor.matmul(
                    ps, wpsb[k][:, m * P:(m + 1) * P], yts[k],
                    start=(k == 0), stop=(k == 3))
            ob = obp.tile([P, 512], F32, tag="ob", name="ob")
            nc.vector.tensor_copy(out=ob, in_=ps)
            nc.sync.dma_start(out=outT[m * P:(m + 1) * P, isl], in_=ob)

        # ---------- schedule ----------
        # round 0: QK of tb0, then ALL gate blocks grouped (one ACT table
        # swap total before the exp stream starts), then V of tb0.
        for m in range(8):
            qk_block(0, m)
        for tb in range(NTB):
            for m in range(4):
                gate_block(tb, m)
        for mt in range(4):
            v_block(0, mt)
        # rounds 1..4: attention it r-1 interleaved with tb r's QK/V blocks
        # and with the projection of it r-2 (lagged one round so the
        # ACT-bound late i-tiles get ACT-free PE filler); round 5: proj it3.
        all_yts = {}
        for r in range(1, 5):
            it = r - 1
            r8 = rrp.tile([P, 1024], F32, tag="r8", name="r8")
            yts = []
            for p in range(4):
                if r <= 3:
                    qk_block(r, 2 * p)
                    qk_block(r, 2 * p + 1)
                yts.append(attn_p(it, p, r8))
                if r >= 2:
                    proj_m(it - 1, all_yts[it - 1], 2 * p)
                    proj_m(it - 1, all_yts[it - 1], 2 * p + 1)
            all_yts[it] = yts
            if r <= 3:
                for mt in range(4):
                    v_block(r, mt)
        for m in range(8):
            proj_m(3, all_yts[3], m)

    nc.compile()
    return nc


def make_in_maps(x, w_attn, w_proj, w_gate, b_gate):
    bf = ml_dtypes.bfloat16
    umask_np = np.triu(np.ones((P, P), dtype=np.float32))
    cst = np.ascontiguousarray(
        np.stack([umask_np, np.zeros((P, P), np.float32)])).astype(bf)
    x = np.asarray(x, dtype=np.float32)
    w_attn = np.asarray(w_attn, dtype=np.float32)
    w_proj = np.asarray(w_proj, dtype=np.float32)
    w_gate = np.asarray(w_gate, dtype=np.float32)
    b_gate = np.asarray(b_gate, dtype=np.float32)
    in_maps = []
    for c in range(8):
        b, g = c // 2, c % 2
        hsl = slice(g * GC, (g + 1) * GC)
        in_maps.append({
            "xT": np.ascontiguousarray(x[b].T).astype(bf),
            "wqk": np.ascontiguousarray(
                np.concatenate([w_attn[:, hsl],
                                w_attn[:, C + g * GC:C + (g + 1) * GC]],
                               axis=1)).astype(bf),
            "wv": np.ascontiguousarray(
                w_attn[:, 2 * C + g * GC:2 * C + (g + 1) * GC]).astype(bf),
            "wg": np.ascontiguousarray(w_gate[:, hsl]).astype(bf),
            "bg": np.ascontiguousarray(b_gate[hsl]),
            "wp": np.ascontiguousarray(w_proj[hsl, :]).astype(bf),
            "cst": cst,
        })
    return in_maps


def kernel(x, w_attn, w_proj, w_gate, b_gate):
    if "nc" not in _NC_CACHE:
        _NC_CACHE["nc"] = _build_nc()
    nc = _NC_CACHE["nc"]

    in_maps = make_in_maps(x, w_attn, w_proj, w_gate, b_gate)
    res = bass_utils.run_bass_kernel_spmd(nc, in_maps, core_ids=list(range(8)))

    out = np.empty((B, T, C), dtype=np.float32)
    for b in range(B):
        acc = res.results[2 * b]["outT"].astype(np.float32)
        acc = acc + res.results[2 * b + 1]["outT"]
        out[b] = acc.T
    return out
